# revision 3
# baseline (speedup 1.0000x reference)
"""Trainium2 Bass kernel for nn_BiLSTM_M_61615600828569 (segment_reduce).

Full computation per batch:
  span_emb = masked-max-pool of token windows   (B,256,768)
  vertex_emb = masked-mean over coref spans     (B,128,768)
  head/tail  = vertex gather by relation        (B,512,768)
  feat = [head, eh, tail, et, head*tail]        (B,512,2344)
  out  = relu(feat @ W1) @ W2 + b2              (B,512,97)

Sharding: data-parallel over batch; 16 batches / 8 cores = 2 per core.
All index work (gather tables, one-hot select matrices, pooling weights)
is precomputed on host; all float math runs on device in bf16 with fp32
PSUM accumulation, in transposed layout (features on partitions) so the
final predict.T has the 97 classes on partitions for a per-partition
bias add.
"""
import numpy as np
import ml_dtypes
from contextlib import ExitStack

import concourse.bacc as bacc
import concourse.tile as tile
from concourse import mybir
from concourse import bass_utils

BF16 = ml_dtypes.bfloat16

B, S, D = 16, 1024, 768
NS, MAXW = 256, 8
V, C = 128, 6
R = 512
REL, HID, DIS = 97, 384, 20
NEG = -1e30

NCORES = 8
NB = B // NCORES          # batches per core = 2
GS = NB * NS              # spans handled per core = 512
NQ = GS // 128            # span row-groups = 4
NIDX = MAXW * GS          # gather rows per core = 4096
SENT_ROWS = NB * S + 1    # staged sentence rows + one NEG row

# W1 row ranges for the five feat blocks -> contraction chunks
FEAT_BLOCKS = [(0, 768), (768, 788), (788, 1556), (1556, 1576), (1576, 2344)]


def _k_chunks():
    """(row0, rows) for each contraction chunk of W1 (<=128 rows each)."""
    out = []
    for r0, r1 in FEAT_BLOCKS:
        r = r0
        while r < r1:
            n = min(128, r1 - r)
            out.append((r, n))
            r += n
    return out


K_CHUNKS = _k_chunks()  # 6 + 1 + 6 + 1 + 6 = 20 chunks


def _patch_drain_and_barrier():
    """Walrus rejects >1 explicit sync wait on a Drain (TPB_CTRL), but Tile's
    tail drain waits on every used proc sem at once. Emit one single-wait
    drain per proc instead; the final drain then needs no waits."""
    import concourse.tile as tile_mod
    from concourse.vector_clock import VectorClock, ScopedClock

    if getattr(tile_mod.TileContext, "_ant_drain_patched", False):
        return

    def _patched(self, tick_clock, wait_clock):
        full = tick_clock.global_clock
        n = len(full)
        for p in [q for q in range(n) if full[q] > 0]:
            vec = [full[q] if q == p else 0 for q in range(n)]
            d = self.nc.sync.drain()
            wait_clock.add_sem_waits(d.ins, ScopedClock({None: VectorClock(vec)}))
        self.nc.sync.drain()
        self.nc.all_engine_barrier()
        popped = self.nc._tile_sem_poison_stack.pop()
        assert popped is self._sem_poison
        self.nc.clear_and_free_semaphores(list(self.sems.allocated().values()))
        self.nc.all_engine_barrier()

    tile_mod.TileContext._drain_and_barrier = _patched
    tile_mod.TileContext._ant_drain_patched = True


_patch_drain_and_barrier()

_NC_CACHE = None


def _build():
    """One-core program; SPMD-replicated across the 8 cores."""
    bf = mybir.dt.bfloat16
    f32 = mybir.dt.float32
    AF = mybir.ActivationFunctionType

    nc = bacc.Bacc("TRN2", target_bir_lowering=False, debug=False, num_devices=1)

    sent = nc.dram_tensor("sent", (SENT_ROWS, D), bf, kind="ExternalInput")
    gidx = nc.dram_tensor("gidx", (128, NIDX // 16), mybir.dt.int16, kind="ExternalInput")
    poolt = nc.dram_tensor("poolt", (NB, 128, 2, V), bf, kind="ExternalInput")
    invcnt = nc.dram_tensor("invcnt", (V, NB), f32, kind="ExternalInput")
    hsel = nc.dram_tensor("hsel", (NB, V, R), bf, kind="ExternalInput")
    tsel = nc.dram_tensor("tsel", (NB, V, R), bf, kind="ExternalInput")
    eht = nc.dram_tensor("eht", (NB, DIS, R), bf, kind="ExternalInput")
    ett = nc.dram_tensor("ett", (NB, DIS, R), bf, kind="ExternalInput")
    w1 = nc.dram_tensor("w1", (2 * (D + DIS) + D, HID), bf, kind="ExternalInput")
    w2 = nc.dram_tensor("w2", (HID, REL), bf, kind="ExternalInput")
    b2t = nc.dram_tensor("b2t", (REL, 1), f32, kind="ExternalInput")
    outd = nc.dram_tensor("outd", (NB, REL, R), f32, kind="ExternalOutput")

    with tile.TileContext(nc) as tc, ExitStack() as ctx:
        consts = ctx.enter_context(tc.tile_pool(name="consts", bufs=1))
        work = ctx.enter_context(tc.tile_pool(name="work", bufs=1))
        perb = ctx.enter_context(tc.tile_pool(name="perb", bufs=2))
        psums = ctx.enter_context(tc.tile_pool(name="psums", bufs=1, space="PSUM"))

        # ---- constants ----
        w1_tiles = []
        for r0, rows in K_CHUNKS:
            w1_t = consts.tile([rows, HID], bf, name=f"w1c_{r0}")
            nc.sync.dma_start(out=w1_t[:], in_=w1.ap()[r0 : r0 + rows, :])
            w1_tiles.append(w1_t)
        w2_tiles = []
        for kc in range(HID // 128):
            w2_t = consts.tile([128, REL], bf, name=f"w2c_{kc}")
            nc.sync.dma_start(out=w2_t[:], in_=w2.ap()[kc * 128 : (kc + 1) * 128, :])
            w2_tiles.append(w2_t)
        b2_t = consts.tile([REL, 1], f32)
        nc.sync.dma_start(out=b2_t[:], in_=b2t.ap())
        inv_t = consts.tile([V, NB], f32)
        nc.sync.dma_start(out=inv_t[:], in_=invcnt.ap())
        idx_t = consts.tile([128, NIDX // 16], mybir.dt.int16)
        nc.gpsimd.dma_start(out=idx_t[:], in_=gidx.ap())

        # ---- span gather + max pool (both batches at once) ----
        # row i = k*GS + s  ->  T[s%128, 4k + s//128, :]
        gat = work.tile([128, NIDX // 128, D], bf)
        nc.gpsimd.dma_gather(
            out_ap=gat[:],
            in_ap=sent.ap(),
            idxs_ap=idx_t[:],
            num_idxs=NIDX,
            num_idxs_reg=NIDX,
            elem_size=D,
            single_packet=False,
        )
        g4 = gat[:].rearrange("p (k q) d -> p k (q d)", k=MAXW, q=NQ)
        m1 = work.tile([128, 4, NQ * D], bf)
        nc.vector.tensor_tensor(out=m1[:], in0=g4[:, 0:4, :], in1=g4[:, 4:8, :], op=mybir.AluOpType.max)
        m2 = work.tile([128, 2, NQ * D], bf)
        nc.vector.tensor_tensor(out=m2[:], in0=m1[:, 0:2, :], in1=m1[:, 2:4, :], op=mybir.AluOpType.max)
        sem = work.tile([128, NQ, D], bf)  # [p, q, :] = span_emb[q*128+p]
        nc.vector.tensor_tensor(
            out=sem[:].rearrange("p q d -> p (q d)"),
            in0=m2[:, 0, :], in1=m2[:, 1, :], op=mybir.AluOpType.max,
        )

        for b in range(NB):
            # ---- vertex mean-pool: V_emb = poolT.T @ span_emb, scaled 1/cnt ----
            pt = perb.tile([128, 2, V], bf, tag="pt")
            nc.sync.dma_start(out=pt[:], in_=poolt.ap()[b])
            ps_v = psums.tile([V, D], mybir.dt.float32, space="PSUM", tag="ps_v")
            for cc in range(2):
                for n0, nsz in ((0, 512), (512, 256)):
                    nc.tensor.matmul(
                        ps_v[:, n0 : n0 + nsz],
                        lhsT=pt[:, cc, :],
                        rhs=sem[:, 2 * b + cc, n0 : n0 + nsz],
                        start=(cc == 0),
                        stop=(cc == 1),
                    )
            v_sb = perb.tile([V, D], bf, tag="v_sb")
            nc.scalar.activation(v_sb[:], ps_v[:], AF.Copy, scale=inv_t[:, b : b + 1])

            # ---- head/tail select: headT[m] = V_emb[:, m-chunk].T @ onehot ----
            hs = perb.tile([V, R], bf, tag="hs")
            nc.sync.dma_start(out=hs[:], in_=hsel.ap()[b])
            ts = perb.tile([V, R], bf, tag="ts")
            nc.sync.dma_start(out=ts[:], in_=tsel.ap()[b])
            head_t = perb.tile([128, 6, R], bf, tag="head_t")
            tail_t = perb.tile([128, 6, R], bf, tag="tail_t")
            for m in range(6):
                ps_h = psums.tile([128, R], mybir.dt.float32, space="PSUM", tag="ps_h")
                nc.tensor.matmul(ps_h[:], lhsT=v_sb[:, m * 128 : (m + 1) * 128], rhs=hs[:], start=True, stop=True)
                nc.any.tensor_copy(head_t[:, m, :], ps_h[:])
                ps_t = psums.tile([128, R], mybir.dt.float32, space="PSUM", tag="ps_t")
                nc.tensor.matmul(ps_t[:], lhsT=v_sb[:, m * 128 : (m + 1) * 128], rhs=ts[:], start=True, stop=True)
                nc.any.tensor_copy(tail_t[:, m, :], ps_t[:])
            prod_t = perb.tile([128, 6, R], bf, tag="prod_t")
            nc.vector.tensor_tensor(out=prod_t[:], in0=head_t[:], in1=tail_t[:], op=mybir.AluOpType.mult)

            eh_t = perb.tile([DIS, R], bf, tag="eh_t")
            nc.sync.dma_start(out=eh_t[:], in_=eht.ap()[b])
            et_t = perb.tile([DIS, R], bf, tag="et_t")
            nc.sync.dma_start(out=et_t[:], in_=ett.ap()[b])

            # featT chunks in W1-row order
            rhs_chunks = [head_t[:, m, :] for m in range(6)]
            rhs_chunks.append(eh_t[:])
            rhs_chunks += [tail_t[:, m, :] for m in range(6)]
            rhs_chunks.append(et_t[:])
            rhs_chunks += [prod_t[:, m, :] for m in range(6)]

            # ---- hiddenT = W1.T @ featT (+relu) ----
            ps_hid = psums.tile([128, 3, R], mybir.dt.float32, space="PSUM", tag="ps_hid")
            for m3 in range(3):
                for i, rhs_ap in enumerate(rhs_chunks):
                    nc.tensor.matmul(
                        ps_hid[:, m3, :],
                        lhsT=w1_tiles[i][:, m3 * 128 : (m3 + 1) * 128],
                        rhs=rhs_ap,
                        start=(i == 0),
                        stop=(i == len(rhs_chunks) - 1),
                    )
            hid_t = perb.tile([128, 3, R], bf, tag="hid_t")
            for m3 in range(3):
                nc.scalar.activation(hid_t[:, m3, :], ps_hid[:, m3, :], AF.Relu)

            # ---- predictT = W2.T @ hiddenT + b2 ----
            ps_o = psums.tile([REL, R], mybir.dt.float32, space="PSUM", tag="ps_o")
            for kc in range(3):
                nc.tensor.matmul(
                    ps_o[:], lhsT=w2_tiles[kc][:], rhs=hid_t[:, kc, :],
                    start=(kc == 0), stop=(kc == 2),
                )
            out_sb = perb.tile([REL, R], f32, tag="out_sb")
            nc.scalar.activation(out_sb[:], ps_o[:], AF.Identity, bias=b2_t[:, 0:1])
            nc.sync.dma_start(out=outd.ap()[b], in_=out_sb[:])

    nc.compile()
    return nc


def _prep_core(c, sentence_repr, esi, vidx, vmask, ht, dis_h, dis_t, dis_embed_b, w1_b, w2_b, b2_f):
    """Build the per-core input map for batches [c*NB, c*NB+NB)."""
    bs = range(c * NB, c * NB + NB)

    sent = np.empty((SENT_ROWS, D), dtype=BF16)
    for j, b in enumerate(bs):
        sent[j * S : (j + 1) * S] = sentence_repr[b].astype(BF16)
    sent[NB * S] = BF16(NEG)

    # gather table: row i = k*GS + s; invalid (k > width) -> NEG row
    starts = np.concatenate([esi[b, :, 0] for b in bs])          # (GS,)
    widths = np.concatenate([esi[b, :, 1] - esi[b, :, 0] for b in bs])
    base = starts + (np.arange(GS) // NS) * S                    # batch-local row base
    k = np.arange(MAXW)[:, None]                                 # (8,1)
    idx = np.where(k <= widths[None, :], base[None, :] + k, NB * S)  # (8, GS)
    flat = idx.reshape(-1).astype(np.int16)                      # i = k*GS + s
    gidx = np.tile(flat.reshape(-1, 16).T, (8, 1)).copy()        # (128, NIDX/16)

    poolt = np.zeros((NB, 128, 2, V), dtype=BF16)
    invcnt = np.zeros((V, NB), dtype=np.float32)
    hsel = np.zeros((NB, V, R), dtype=BF16)
    tsel = np.zeros((NB, V, R), dtype=BF16)
    eht = np.empty((NB, DIS, R), dtype=BF16)
    ett = np.empty((NB, DIS, R), dtype=BF16)
    for j, b in enumerate(bs):
        pt = np.zeros((NS, V), dtype=np.float32)
        np.add.at(pt, (vidx[b].ravel(), np.repeat(np.arange(V), C)), vmask[b].ravel().astype(np.float32))
        poolt[j] = pt.reshape(2, 128, V).transpose(1, 0, 2).astype(BF16)
        invcnt[:, j] = 1.0 / np.maximum(vmask[b].sum(axis=1).astype(np.float32), 1.0)
        hsel[j, ht[b, :, 0], np.arange(R)] = BF16(1.0)
        tsel[j, ht[b, :, 1], np.arange(R)] = BF16(1.0)
        eht[j] = dis_embed_b[dis_h[b]].T
        ett[j] = dis_embed_b[dis_t[b]].T

    return dict(
        sent=sent, gidx=gidx, poolt=poolt, invcnt=invcnt,
        hsel=hsel, tsel=tsel, eht=eht, ett=ett,
        w1=w1_b, w2=w2_b, b2t=b2_f,
    )


def run(trace=False, **inputs):
    global _NC_CACHE
    sentence_repr = np.asarray(inputs["sentence_repr"], dtype=np.float32)
    esi = np.asarray(inputs["entity_span_indices"]).astype(np.int64)
    vidx = np.asarray(inputs["vertex_indices"]).astype(np.int64)
    vmask = np.asarray(inputs["vertex_indices_mask"]).astype(np.int64)
    ht = np.asarray(inputs["head_tail_indices"]).astype(np.int64)
    dis_h = np.asarray(inputs["dis_h_2_t"]).astype(np.int64)
    dis_t = np.asarray(inputs["dis_t_2_h"]).astype(np.int64)
    dis_embed = np.asarray(inputs["dis_embed"], dtype=np.float32)
    w1 = np.asarray(inputs["W1"], dtype=np.float32)
    w2 = np.asarray(inputs["W2"], dtype=np.float32)
    b2 = np.asarray(inputs["b2"], dtype=np.float32)

    dis_embed_b = dis_embed.astype(BF16)
    w1_b = w1.astype(BF16)
    w2_b = w2.astype(BF16)
    b2_f = b2.reshape(REL, 1).astype(np.float32)

    in_maps = [
        _prep_core(c, sentence_repr, esi, vidx, vmask, ht, dis_h, dis_t,
                   dis_embed_b, w1_b, w2_b, b2_f)
        for c in range(NCORES)
    ]

    if _NC_CACHE is None:
        _NC_CACHE = _build()

    res = bass_utils.run_bass_kernel_spmd(
        _NC_CACHE, in_maps, core_ids=list(range(NCORES)), trace=trace
    )

    out = np.empty((B, R, REL), dtype=np.float32)
    for c in range(NCORES):
        o = np.asarray(res.results[c]["outd"], dtype=np.float32)  # (NB, REL, R)
        for j in range(NB):
            out[c * NB + j] = o[j].T
    return out, res


def kernel(**inputs):
    out, _ = run(**inputs)
    return out


# revision 6
# speedup vs baseline: 1.3590x; 1.3590x over previous
"""Trainium2 Bass kernel for nn_BiLSTM_M_61615600828569 (segment_reduce).

Full computation per batch:
  span_emb = masked-max-pool of token windows   (B,256,768)
  vertex_emb = masked-mean over coref spans     (B,128,768)
  head/tail  = vertex gather by relation        (B,512,768)
  feat = [head, eh, tail, et, head*tail]        (B,512,2344)
  out  = relu(feat @ W1) @ W2 + b2              (B,512,97)

Sharding: data-parallel over batch; 16 batches / 8 cores = 2 per core.
All index work (gather tables, one-hot select matrices, pooling weights)
is precomputed on host; all float math runs on device in bf16 with fp32
PSUM accumulation, in transposed layout (features on partitions) so the
final predict.T has the 97 classes on partitions for a per-partition
bias add.

Span pooling: token rows are fetched with dma_gather in four span-group
chunks (pipelines Q7 descriptor generation against DMA transfer and lets
batch-0 compute start after two chunks); rows past a span's width are
redirected to a staged -1e30 row, so a strided DVE max-tree needs no
masking. W1 is zero-padded to 20 uniform 128-row contraction chunks so
the eh/et blocks ride the same accumulation loop (their rhs rows past
row 19 are garbage multiplied by zero weights).
"""
import numpy as np
import ml_dtypes
from contextlib import ExitStack

import concourse.bacc as bacc
import concourse.tile as tile
from concourse import mybir
from concourse import bass_utils

BF16 = ml_dtypes.bfloat16

B, S, D = 16, 1024, 768
NS, MAXW = 256, 8
V, C = 128, 6
R = 512
REL, HID, DIS = 97, 384, 20
NEG = -1e30

NCORES = 8
NB = B // NCORES          # batches per core = 2
GS = NB * NS              # spans per core = 512
NQ = GS // 128            # span groups = 4
NIDXQ = MAXW * 128        # gather rows per span group = 1024
SENT_ROWS = NB * S + 1    # staged sentence rows + one NEG row
NKC = 20                  # uniform 128-row W1 contraction chunks
W1PAD = NKC * 128         # zero-padded W1 rows = 2560

# W1 row ranges of the five feat blocks, in order
FEAT_BLOCKS = [(0, 768), (768, 788), (788, 1556), (1556, 1576), (1576, 2344)]


def _patch_drain_and_barrier():
    """Walrus rejects >1 explicit sync wait on a Drain (TPB_CTRL), but Tile's
    tail drain waits on every used proc sem at once. Emit one single-wait
    drain per proc instead; the final drain then needs no waits."""
    import concourse.tile as tile_mod
    from concourse.vector_clock import VectorClock, ScopedClock

    if getattr(tile_mod.TileContext, "_ant_drain_patched", False):
        return

    def _patched(self, tick_clock, wait_clock):
        full = tick_clock.global_clock
        n = len(full)
        for p in [q for q in range(n) if full[q] > 0]:
            vec = [full[q] if q == p else 0 for q in range(n)]
            d = self.nc.sync.drain()
            wait_clock.add_sem_waits(d.ins, ScopedClock({None: VectorClock(vec)}))
        self.nc.sync.drain()
        self.nc.all_engine_barrier()
        popped = self.nc._tile_sem_poison_stack.pop()
        assert popped is self._sem_poison
        self.nc.clear_and_free_semaphores(list(self.sems.allocated().values()))
        self.nc.all_engine_barrier()

    tile_mod.TileContext._drain_and_barrier = _patched
    tile_mod.TileContext._ant_drain_patched = True


_patch_drain_and_barrier()

_NC_CACHE = None


def _build():
    """One-core program; SPMD-replicated across the 8 cores."""
    bf = mybir.dt.bfloat16
    f32 = mybir.dt.float32
    AF = mybir.ActivationFunctionType
    MAX = mybir.AluOpType.max

    nc = bacc.Bacc("TRN2", target_bir_lowering=False, debug=False, num_devices=1)

    sent = nc.dram_tensor("sent", (SENT_ROWS, D), bf, kind="ExternalInput")
    gidx = nc.dram_tensor("gidx", (128, NQ, NIDXQ // 16), mybir.dt.int16, kind="ExternalInput")
    poolt = nc.dram_tensor("poolt", (128, NB, 2, V), bf, kind="ExternalInput")
    invcnt = nc.dram_tensor("invcnt", (V, NB), f32, kind="ExternalInput")
    hsel = nc.dram_tensor("hsel", (V, NB, R), bf, kind="ExternalInput")
    tsel = nc.dram_tensor("tsel", (V, NB, R), bf, kind="ExternalInput")
    eht = nc.dram_tensor("eht", (DIS, NB, R), bf, kind="ExternalInput")
    ett = nc.dram_tensor("ett", (DIS, NB, R), bf, kind="ExternalInput")
    w1 = nc.dram_tensor("w1", (128, NKC, HID), bf, kind="ExternalInput")
    w2 = nc.dram_tensor("w2", (128, HID // 128, REL), bf, kind="ExternalInput")
    b2t = nc.dram_tensor("b2t", (REL, 1), f32, kind="ExternalInput")
    outd = nc.dram_tensor("outd", (NB, REL, R), f32, kind="ExternalOutput")

    with tile.TileContext(nc) as tc, ExitStack() as ctx:
        consts = ctx.enter_context(tc.tile_pool(name="consts", bufs=1))
        work = ctx.enter_context(tc.tile_pool(name="work", bufs=1))
        perb = ctx.enter_context(tc.tile_pool(name="perb", bufs=2))
        psums = ctx.enter_context(tc.tile_pool(name="psums", bufs=1, space="PSUM"))

        def psum_tile(name):
            return psums.tile([128, R], mybir.dt.float32, space="PSUM",
                              tag="ps", bufs=8, name=name)

        # ---- constant loads (one DMA each) ----
        idx_t = consts.tile([128, NQ, NIDXQ // 16], mybir.dt.int16)
        nc.gpsimd.dma_start(out=idx_t[:], in_=gidx.ap())
        w1_t = consts.tile([128, NKC, HID], bf)
        nc.sync.dma_start(out=w1_t[:], in_=w1.ap())
        w2_t = consts.tile([128, HID // 128, REL], bf)
        nc.sync.dma_start(out=w2_t[:], in_=w2.ap())
        b2_t = consts.tile([REL, 1], f32)
        nc.sync.dma_start(out=b2_t[:], in_=b2t.ap())
        inv_t = consts.tile([V, NB], f32)
        nc.sync.dma_start(out=inv_t[:], in_=invcnt.ap())
        pt_t = consts.tile([128, NB, 2, V], bf)
        nc.sync.dma_start(out=pt_t[:], in_=poolt.ap())
        hs_t = consts.tile([V, NB, R], bf)
        nc.sync.dma_start(out=hs_t[:], in_=hsel.ap())
        ts_t = consts.tile([V, NB, R], bf)
        nc.sync.dma_start(out=ts_t[:], in_=tsel.ap())
        eh_t = consts.tile([128, NB, R], bf)
        nc.vector.memset(eh_t[:], 0.0)
        nc.sync.dma_start(out=eh_t[:DIS], in_=eht.ap())
        et_t = consts.tile([128, NB, R], bf)
        nc.vector.memset(et_t[:], 0.0)
        nc.sync.dma_start(out=et_t[:DIS], in_=ett.ap())

        # ---- span gather + max-tree, one span group (128 spans) at a time ----
        # group q: gather row i = k*128 + p  ->  gq[p, k, :]
        sem_q = []  # sem_q[q][p, :] = span_emb[q*128 + p]
        for q in range(NQ):
            gq = work.tile([128, MAXW, D], bf, name=f"g_{q}", tag=f"g_{q}")
            nc.gpsimd.dma_gather(
                out_ap=gq[:],
                in_ap=sent.ap(),
                idxs_ap=idx_t[:, q, :],
                num_idxs=NIDXQ,
                num_idxs_reg=NIDXQ,
                elem_size=D,
                single_packet=False,
            )
            m1 = work.tile([128, 4, D], bf, name=f"m1_{q}", tag=f"m1_{q}")
            nc.vector.tensor_tensor(out=m1[:], in0=gq[:, 0:4, :], in1=gq[:, 4:8, :], op=MAX)
            m2 = work.tile([128, 2, D], bf, name=f"m2_{q}", tag=f"m2_{q}")
            nc.vector.tensor_tensor(out=m2[:], in0=m1[:, 0:2, :], in1=m1[:, 2:4, :], op=MAX)
            sq = work.tile([128, D], bf, name=f"sem_{q}", tag=f"sem_{q}")
            nc.vector.tensor_tensor(out=sq[:], in0=m2[:, 0, :], in1=m2[:, 1, :], op=MAX)
            sem_q.append(sq)

            if q % 2 == 1:
                b = q // 2
                # ---- vertex mean-pool: V_emb = poolT.T @ span_emb ----
                ps_v = [psum_tile("ps_v1"), psum_tile("ps_v2")]
                for cc in range(2):
                    for ni, (n0, nsz) in enumerate(((0, 512), (512, 256))):
                        nc.tensor.matmul(
                            ps_v[ni][:, :nsz],
                            lhsT=pt_t[:, b, cc, :],
                            rhs=sem_q[2 * b + cc][:, n0 : n0 + nsz],
                            start=(cc == 0),
                            stop=(cc == 1),
                        )
                v_sb = perb.tile([V, D], bf, tag="v_sb")
                nc.scalar.activation(v_sb[:, 0:512], ps_v[0][:, :], AF.Copy, scale=inv_t[:, b : b + 1])
                nc.scalar.activation(v_sb[:, 512:768], ps_v[1][:, :256], AF.Copy, scale=inv_t[:, b : b + 1])

                # ---- head/tail select ----
                head_t = perb.tile([128, 6, R], bf, tag="head_t")
                tail_t = perb.tile([128, 6, R], bf, tag="tail_t")
                for m in range(6):
                    ps_h = psum_tile("ps_h")
                    nc.tensor.matmul(ps_h[:], lhsT=v_sb[:, m * 128 : (m + 1) * 128],
                                     rhs=hs_t[:, b, :], start=True, stop=True)
                    nc.any.tensor_copy(head_t[:, m, :], ps_h[:])
                    ps_t2 = psum_tile("ps_t2")
                    nc.tensor.matmul(ps_t2[:], lhsT=v_sb[:, m * 128 : (m + 1) * 128],
                                     rhs=ts_t[:, b, :], start=True, stop=True)
                    nc.any.tensor_copy(tail_t[:, m, :], ps_t2[:])
                prod_t = perb.tile([128, 6, R], bf, tag="prod_t")
                nc.vector.tensor_tensor(out=prod_t[:], in0=head_t[:], in1=tail_t[:],
                                        op=mybir.AluOpType.mult)

                # featT chunks in (padded) W1 row order
                rhs_chunks = [head_t[:, m, :] for m in range(6)]
                rhs_chunks.append(eh_t[:, b, :])
                rhs_chunks += [tail_t[:, m, :] for m in range(6)]
                rhs_chunks.append(et_t[:, b, :])
                rhs_chunks += [prod_t[:, m, :] for m in range(6)]

                # ---- hiddenT = W1.T @ featT (+relu) ----
                hid_t = perb.tile([128, 3, R], bf, tag="hid_t")
                for m3 in range(3):
                    ps_hid = psum_tile("ps_hid")
                    for i, rhs_ap in enumerate(rhs_chunks):
                        nc.tensor.matmul(
                            ps_hid[:],
                            lhsT=w1_t[:, i, m3 * 128 : (m3 + 1) * 128],
                            rhs=rhs_ap,
                            start=(i == 0),
                            stop=(i == NKC - 1),
                        )
                    nc.scalar.activation(hid_t[:, m3, :], ps_hid[:], AF.Relu)

                # ---- predictT = W2.T @ hiddenT + b2 ----
                ps_o = psum_tile("ps_o")
                for kc in range(3):
                    nc.tensor.matmul(
                        ps_o[:REL, :], lhsT=w2_t[:, kc, :], rhs=hid_t[:, kc, :],
                        start=(kc == 0), stop=(kc == 2),
                    )
                out_sb = perb.tile([REL, R], f32, tag="out_sb")
                nc.scalar.activation(out_sb[:], ps_o[:REL, :], AF.Identity, bias=b2_t[:, 0:1])
                nc.sync.dma_start(out=outd.ap()[b], in_=out_sb[:])

    nc.compile()
    return nc


def _prep_core(c, sentence_repr, esi, vidx, vmask, ht, dis_h, dis_t,
               dis_embed_b, w1_p, w2_p, b2_f):
    """Build the per-core input map for batches [c*NB, c*NB+NB)."""
    bs = range(c * NB, c * NB + NB)

    sent = np.empty((SENT_ROWS, D), dtype=BF16)
    for j, b in enumerate(bs):
        sent[j * S : (j + 1) * S] = sentence_repr[b].astype(BF16)
    sent[NB * S] = BF16(NEG)

    # per-group gather tables: group q covers spans q*128..q*128+127;
    # row i = k*128 + p; invalid (k > width) -> NEG row
    starts = np.concatenate([esi[b, :, 0] for b in bs])          # (GS,)
    widths = np.concatenate([esi[b, :, 1] - esi[b, :, 0] for b in bs])
    base = starts + (np.arange(GS) // NS) * S                    # batch-local row base
    k = np.arange(MAXW)[:, None]                                 # (8,1)
    idx = np.where(k <= widths[None, :], base[None, :] + k, NB * S)  # (8, GS)
    gidx = np.empty((128, NQ, NIDXQ // 16), dtype=np.int16)
    for q in range(NQ):
        flat = idx[:, q * 128 : (q + 1) * 128].reshape(-1).astype(np.int16)  # i = k*128+p
        gidx[:, q, :] = np.tile(flat.reshape(-1, 16).T, (8, 1))

    poolt = np.zeros((128, NB, 2, V), dtype=BF16)
    invcnt = np.zeros((V, NB), dtype=np.float32)
    hsel = np.zeros((V, NB, R), dtype=BF16)
    tsel = np.zeros((V, NB, R), dtype=BF16)
    eht = np.empty((DIS, NB, R), dtype=BF16)
    ett = np.empty((DIS, NB, R), dtype=BF16)
    for j, b in enumerate(bs):
        pt = np.zeros((NS, V), dtype=np.float32)
        np.add.at(pt, (vidx[b].ravel(), np.repeat(np.arange(V), C)), vmask[b].ravel().astype(np.float32))
        poolt[:, j] = pt.reshape(2, 128, V).transpose(1, 0, 2).astype(BF16)
        invcnt[:, j] = 1.0 / np.maximum(vmask[b].sum(axis=1).astype(np.float32), 1.0)
        hsel[ht[b, :, 0], j, np.arange(R)] = BF16(1.0)
        tsel[ht[b, :, 1], j, np.arange(R)] = BF16(1.0)
        eht[:, j] = dis_embed_b[dis_h[b]].T
        ett[:, j] = dis_embed_b[dis_t[b]].T

    return dict(
        sent=sent, gidx=gidx, poolt=poolt, invcnt=invcnt,
        hsel=hsel, tsel=tsel, eht=eht, ett=ett,
        w1=w1_p, w2=w2_p, b2t=b2_f,
    )


def run(trace=False, **inputs):
    global _NC_CACHE
    sentence_repr = np.asarray(inputs["sentence_repr"], dtype=np.float32)
    esi = np.asarray(inputs["entity_span_indices"]).astype(np.int64)
    vidx = np.asarray(inputs["vertex_indices"]).astype(np.int64)
    vmask = np.asarray(inputs["vertex_indices_mask"]).astype(np.int64)
    ht = np.asarray(inputs["head_tail_indices"]).astype(np.int64)
    dis_h = np.asarray(inputs["dis_h_2_t"]).astype(np.int64)
    dis_t = np.asarray(inputs["dis_t_2_h"]).astype(np.int64)
    dis_embed = np.asarray(inputs["dis_embed"], dtype=np.float32)
    w1 = np.asarray(inputs["W1"], dtype=np.float32)
    w2 = np.asarray(inputs["W2"], dtype=np.float32)
    b2 = np.asarray(inputs["b2"], dtype=np.float32)

    dis_embed_b = dis_embed.astype(BF16)
    # zero-pad W1 blocks to 20 uniform 128-row chunks, laid out [p, chunk, :]
    w1_pad = np.zeros((W1PAD, HID), dtype=BF16)
    dst = 0
    for r0, r1 in FEAT_BLOCKS:
        rows = r1 - r0
        nch = (rows + 127) // 128
        for i in range(nch):
            a = r0 + i * 128
            n = min(128, r1 - a)
            w1_pad[dst : dst + n] = w1[a : a + n].astype(BF16)
            dst += 128
    assert dst == W1PAD
    w1_p = np.ascontiguousarray(w1_pad.reshape(NKC, 128, HID).transpose(1, 0, 2))
    w2_p = np.ascontiguousarray(w2.astype(BF16).reshape(HID // 128, 128, REL).transpose(1, 0, 2))
    b2_f = b2.reshape(REL, 1).astype(np.float32)

    in_maps = [
        _prep_core(c, sentence_repr, esi, vidx, vmask, ht, dis_h, dis_t,
                   dis_embed_b, w1_p, w2_p, b2_f)
        for c in range(NCORES)
    ]

    if _NC_CACHE is None:
        _NC_CACHE = _build()

    res = bass_utils.run_bass_kernel_spmd(
        _NC_CACHE, in_maps, core_ids=list(range(NCORES)), trace=trace
    )

    out = np.empty((B, R, REL), dtype=np.float32)
    for c in range(NCORES):
        o = np.asarray(res.results[c]["outd"], dtype=np.float32)  # (NB, REL, R)
        for j in range(NB):
            out[c * NB + j] = o[j].T
    return out, res


def kernel(**inputs):
    out, _ = run(**inputs)
    return out


# revision 8
# speedup vs baseline: 1.4462x; 1.0641x over previous
"""Trainium2 Bass kernel for nn_BiLSTM_M_61615600828569 (segment_reduce).

Full computation per batch:
  span_emb = masked-max-pool of token windows   (B,256,768)
  vertex_emb = masked-mean over coref spans     (B,128,768)
  head/tail  = vertex gather by relation        (B,512,768)
  feat = [head, eh, tail, et, head*tail]        (B,512,2344)
  out  = relu(feat @ W1) @ W2 + b2              (B,512,97)

Sharding: data-parallel over batch; 16 batches / 8 cores = 2 per core.
All index work (gather tables, one-hot select matrices, pooling weights)
is precomputed on host; all float math runs on device in bf16 with fp32
PSUM accumulation, in transposed layout (features on partitions) so the
final predict.T has the 97 classes on partitions for a per-partition
bias add.

Span pooling: token rows are fetched with dma_gather at PAIR granularity
(elem = 2 overlapping rows via elem_step) — 4 passes whose pair base is
start + min(2j, w-1) jointly cover [start, start+w] for any width w>=1;
zero-width spans keep pass 0's (start, start+1) pair and a broadcast
additive mask (-2e30) kills the invalid second row on the DVE. Passes are
split per batch so batch-0 compute starts while batch-1 still gathers.
W1 is zero-padded to 20 uniform 128-row contraction chunks so the eh/et
blocks ride the same accumulation loop (their rhs rows past row 19 are
zeros times zero weights).
"""
import numpy as np
import ml_dtypes
from contextlib import ExitStack

import concourse.bass as bass
import concourse.bacc as bacc
import concourse.tile as tile
from concourse import mybir
from concourse import bass_utils

BF16 = ml_dtypes.bfloat16

B, S, D = 16, 1024, 768
NS, MAXW = 256, 8
V, C = 128, 6
R = 512
REL, HID, DIS = 97, 384, 20
NEG = -1e30

NCORES = 8
NB = B // NCORES          # batches per core = 2
GS = NB * NS              # spans per core = 512
NQ = GS // 128            # span groups = 4
NPASS = 4                 # pair passes per batch
SENT_ROWS = NB * S + 2    # staged sentence rows + two NEG rows (NEG pair)
NEGROW = NB * S
NKC = 20                  # uniform 128-row W1 contraction chunks
W1PAD = NKC * 128

FEAT_BLOCKS = [(0, 768), (768, 788), (788, 1556), (1556, 1576), (1576, 2344)]


def _patch_drain_and_barrier():
    """Walrus rejects >1 explicit sync wait on a Drain (TPB_CTRL), but Tile's
    tail drain waits on every used proc sem at once. Emit one single-wait
    drain per proc instead; the final drain then needs no waits."""
    import concourse.tile as tile_mod
    from concourse.vector_clock import VectorClock, ScopedClock

    if getattr(tile_mod.TileContext, "_ant_drain_patched", False):
        return

    def _patched(self, tick_clock, wait_clock):
        full = tick_clock.global_clock
        n = len(full)
        for p in [q for q in range(n) if full[q] > 0]:
            vec = [full[q] if q == p else 0 for q in range(n)]
            d = self.nc.sync.drain()
            wait_clock.add_sem_waits(d.ins, ScopedClock({None: VectorClock(vec)}))
        self.nc.sync.drain()
        self.nc.all_engine_barrier()
        popped = self.nc._tile_sem_poison_stack.pop()
        assert popped is self._sem_poison
        self.nc.clear_and_free_semaphores(list(self.sems.allocated().values()))
        self.nc.all_engine_barrier()

    tile_mod.TileContext._drain_and_barrier = _patched
    tile_mod.TileContext._ant_drain_patched = True


_patch_drain_and_barrier()

_NC_CACHE = None


def _build():
    """One-core program; SPMD-replicated across the 8 cores."""
    bf = mybir.dt.bfloat16
    f32 = mybir.dt.float32
    AF = mybir.ActivationFunctionType
    MAX = mybir.AluOpType.max

    nc = bacc.Bacc("TRN2", target_bir_lowering=False, debug=False, num_devices=1)

    sent = nc.dram_tensor("sent", (SENT_ROWS, D), bf, kind="ExternalInput")
    gidx = nc.dram_tensor("gidx", (128, NB, NPASS, 16), mybir.dt.int16, kind="ExternalInput")
    w0m = nc.dram_tensor("w0m", (128, NQ), bf, kind="ExternalInput")
    poolt = nc.dram_tensor("poolt", (128, NB, 2, V), bf, kind="ExternalInput")
    invcnt = nc.dram_tensor("invcnt", (V, NB), f32, kind="ExternalInput")
    hsel = nc.dram_tensor("hsel", (V, NB, R), bf, kind="ExternalInput")
    tsel = nc.dram_tensor("tsel", (V, NB, R), bf, kind="ExternalInput")
    eht = nc.dram_tensor("eht", (DIS, NB, R), bf, kind="ExternalInput")
    ett = nc.dram_tensor("ett", (DIS, NB, R), bf, kind="ExternalInput")
    w1 = nc.dram_tensor("w1", (128, NKC, HID), bf, kind="ExternalInput")
    w2 = nc.dram_tensor("w2", (128, HID // 128, REL), bf, kind="ExternalInput")
    b2t = nc.dram_tensor("b2t", (REL, 1), f32, kind="ExternalInput")
    outd = nc.dram_tensor("outd", (NB, REL, R), f32, kind="ExternalOutput")

    # overlapping-pair view of the staged sentence: row i -> rows [i, i+1]
    sent_pairs = bass.AP(tensor=sent.ap().tensor, offset=0,
                         ap=[[D, SENT_ROWS - 1], [1, 2 * D]])

    with tile.TileContext(nc) as tc, ExitStack() as ctx:
        consts = ctx.enter_context(tc.tile_pool(name="consts", bufs=1))
        work = ctx.enter_context(tc.tile_pool(name="work", bufs=1))
        perb = ctx.enter_context(tc.tile_pool(name="perb", bufs=2))
        psums = ctx.enter_context(tc.tile_pool(name="psums", bufs=1, space="PSUM"))

        def psum_tile(name):
            return psums.tile([128, R], mybir.dt.float32, space="PSUM",
                              tag="ps", bufs=8, name=name)

        # ---- gather index table first: the Q7 is the gather's serial resource ----
        idx_t = consts.tile([128, NB, NPASS, 16], mybir.dt.int16)
        nc.sync.dma_start(out=idx_t[:], in_=gidx.ap())
        w0m_t = consts.tile([128, NQ], bf)
        nc.sync.dma_start(out=w0m_t[:], in_=w0m.ap())

        # ---- pair gathers: per batch h, 4 passes of 256 pair-descriptors ----
        pair_tiles = [[None] * NPASS for _ in range(NB)]
        for h in range(NB):
            for j in range(NPASS):
                gt = work.tile([128, 2, 2 * D], bf, name=f"gp_{h}_{j}", tag=f"gp_{h}_{j}")
                nc.gpsimd.dma_gather(
                    out_ap=gt[:],
                    in_ap=sent_pairs,
                    idxs_ap=idx_t[:, h, j, :],
                    num_idxs=256,
                    num_idxs_reg=256,
                    elem_size=2 * D,
                    elem_step=D,
                    single_packet=False,
                )
                pair_tiles[h][j] = gt

        # ---- constant loads (one DMA each) ----
        w1_t = consts.tile([128, NKC, HID], bf)
        nc.sync.dma_start(out=w1_t[:], in_=w1.ap())
        w2_t = consts.tile([128, HID // 128, REL], bf)
        nc.sync.dma_start(out=w2_t[:], in_=w2.ap())
        b2_t = consts.tile([REL, 1], f32)
        nc.sync.dma_start(out=b2_t[:], in_=b2t.ap())
        inv_t = consts.tile([V, NB], f32)
        nc.sync.dma_start(out=inv_t[:], in_=invcnt.ap())
        pt_t = consts.tile([128, NB, 2, V], bf)
        nc.sync.dma_start(out=pt_t[:], in_=poolt.ap())
        hs_t = consts.tile([V, NB, R], bf)
        nc.sync.dma_start(out=hs_t[:], in_=hsel.ap())
        ts_t = consts.tile([V, NB, R], bf)
        nc.sync.dma_start(out=ts_t[:], in_=tsel.ap())
        eh_t = consts.tile([128, NB, R], bf)
        nc.vector.memset(eh_t[:], 0.0)
        nc.sync.dma_start(out=eh_t[:DIS], in_=eht.ap())
        et_t = consts.tile([128, NB, R], bf)
        nc.vector.memset(et_t[:], 0.0)
        nc.sync.dma_start(out=et_t[:DIS], in_=ett.ap())

        # ---- max-tree per batch: pairs -> span_emb q-slices ----
        sem_b = []  # sem_b[h][p, cc, :] = span_emb[(2h+cc)*128 + p]
        for h in range(NB):
            pm = []
            for j in range(NPASS):
                gt = pair_tiles[h][j]
                g4 = gt[:].rearrange("p q (r d) -> p q r d", r=2)
                pmj = work.tile([128, 2, D], bf, name=f"pm_{h}_{j}", tag=f"pm_{h}_{j}")
                if j == 0:
                    # kill the invalid second row of zero-width spans
                    fix = work.tile([128, 2, D], bf, name=f"fix_{h}", tag=f"fix_{h}")
                    mask = w0m_t[:, 2 * h : 2 * h + 2].broadcast_to([128, 2, D])
                    nc.vector.tensor_tensor(out=fix[:], in0=g4[:, :, 1, :],
                                            in1=mask, op=mybir.AluOpType.add)
                    nc.vector.tensor_tensor(out=pmj[:], in0=g4[:, :, 0, :], in1=fix[:], op=MAX)
                else:
                    nc.vector.tensor_tensor(out=pmj[:], in0=g4[:, :, 0, :], in1=g4[:, :, 1, :], op=MAX)
                pm.append(pmj)
            t01 = work.tile([128, 2, D], bf, name=f"t01_{h}", tag=f"t01_{h}")
            nc.vector.tensor_tensor(out=t01[:], in0=pm[0][:], in1=pm[1][:], op=MAX)
            sh = work.tile([128, 2, D], bf, name=f"sem_{h}", tag=f"sem_{h}")
            nc.vector.tensor_tensor(out=sh[:], in0=pm[2][:], in1=pm[3][:], op=MAX)
            nc.vector.tensor_tensor(out=sh[:], in0=sh[:], in1=t01[:], op=MAX)
            sem_b.append(sh)

        # ---- per-batch compute, batch-interleaved so the PE stays fed ----
        v_sbs, head_ts, tail_ts, prod_ts, hid_ts = {}, {}, {}, {}, {}
        for b in range(NB):
            ps_v = [psum_tile("ps_v1"), psum_tile("ps_v2")]
            for cc in range(2):
                for ni, (n0, nsz) in enumerate(((0, 512), (512, 256))):
                    nc.tensor.matmul(
                        ps_v[ni][:, :nsz],
                        lhsT=pt_t[:, b, cc, :],
                        rhs=sem_b[b][:, cc, n0 : n0 + nsz],
                        start=(cc == 0),
                        stop=(cc == 1),
                    )
            v_sb = perb.tile([V, D], bf, tag="v_sb")
            nc.scalar.activation(v_sb[:, 0:512], ps_v[0][:, :], AF.Copy, scale=inv_t[:, b : b + 1])
            nc.scalar.activation(v_sb[:, 512:768], ps_v[1][:, :256], AF.Copy, scale=inv_t[:, b : b + 1])
            v_sbs[b] = v_sb

        for b in range(NB):
            head_t = perb.tile([128, 6, R], bf, tag="head_t")
            tail_t = perb.tile([128, 6, R], bf, tag="tail_t")
            for m in range(6):
                ps_h = psum_tile("ps_h")
                nc.tensor.matmul(ps_h[:], lhsT=v_sbs[b][:, m * 128 : (m + 1) * 128],
                                 rhs=hs_t[:, b, :], start=True, stop=True)
                nc.any.tensor_copy(head_t[:, m, :], ps_h[:])
                ps_t2 = psum_tile("ps_t2")
                nc.tensor.matmul(ps_t2[:], lhsT=v_sbs[b][:, m * 128 : (m + 1) * 128],
                                 rhs=ts_t[:, b, :], start=True, stop=True)
                nc.any.tensor_copy(tail_t[:, m, :], ps_t2[:])
            prod_t = perb.tile([128, 6, R], bf, tag="prod_t")
            nc.vector.tensor_tensor(out=prod_t[:], in0=head_t[:], in1=tail_t[:],
                                    op=mybir.AluOpType.mult)
            head_ts[b], tail_ts[b], prod_ts[b] = head_t, tail_t, prod_t

        for b in range(NB):
            rhs_chunks = [head_ts[b][:, m, :] for m in range(6)]
            rhs_chunks.append(eh_t[:, b, :])
            rhs_chunks += [tail_ts[b][:, m, :] for m in range(6)]
            rhs_chunks.append(et_t[:, b, :])
            rhs_chunks += [prod_ts[b][:, m, :] for m in range(6)]

            hid_t = perb.tile([128, 3, R], bf, tag="hid_t")
            for m3 in range(3):
                ps_hid = psum_tile("ps_hid")
                for i, rhs_ap in enumerate(rhs_chunks):
                    nc.tensor.matmul(
                        ps_hid[:],
                        lhsT=w1_t[:, i, m3 * 128 : (m3 + 1) * 128],
                        rhs=rhs_ap,
                        start=(i == 0),
                        stop=(i == NKC - 1),
                    )
                nc.scalar.activation(hid_t[:, m3, :], ps_hid[:], AF.Relu)
            hid_ts[b] = hid_t

        for b in range(NB):
            ps_o = psum_tile("ps_o")
            for kc in range(3):
                nc.tensor.matmul(
                    ps_o[:REL, :], lhsT=w2_t[:, kc, :], rhs=hid_ts[b][:, kc, :],
                    start=(kc == 0), stop=(kc == 2),
                )
            out_sb = perb.tile([REL, R], f32, tag="out_sb")
            nc.scalar.activation(out_sb[:], ps_o[:REL, :], AF.Identity, bias=b2_t[:, 0:1])
            nc.sync.dma_start(out=outd.ap()[b], in_=out_sb[:])

    nc.compile()
    return nc


def _prep_core(c, sentence_repr, esi, vidx, vmask, ht, dis_h, dis_t,
               dis_embed_b, w1_p, w2_p, b2_f):
    """Build the per-core input map for batches [c*NB, c*NB+NB)."""
    bs = range(c * NB, c * NB + NB)

    sent = np.empty((SENT_ROWS, D), dtype=BF16)
    for j, b in enumerate(bs):
        sent[j * S : (j + 1) * S] = sentence_repr[b].astype(BF16)
    sent[NEGROW:] = BF16(NEG)

    # pair-gather tables: batch h, pass j, span i = q*128+p (local);
    # pair base = start + min(2j, w-1); w==0 keeps pass 0 only (masked later)
    starts = np.stack([esi[b, :, 0] for b in bs])                 # (NB, NS)
    widths = np.stack([esi[b, :, 1] - esi[b, :, 0] for b in bs])  # (NB, NS)
    gidx = np.empty((128, NB, NPASS, 16), dtype=np.int16)
    for h in range(NB):
        st, w = starts[h], widths[h]
        for j in range(NPASS):
            base = st + np.minimum(2 * j, np.maximum(w - 1, 0)) + h * S
            if j == 0:
                idx = base                                        # w==0 handled by mask
            else:
                idx = np.where(w >= 1, base, NEGROW)
            flat = idx.astype(np.int16)                           # i = q*128+p order
            gidx[:, h, j, :] = np.tile(flat.reshape(-1, 16).T, (8, 1))

    w0mv = np.zeros((128, NQ), dtype=BF16)
    wq = widths.reshape(NQ, 128)                                   # [q, p]
    w0mv[:, :] = np.where(wq.T == 0, BF16(-2e30), BF16(0.0))

    poolt = np.zeros((128, NB, 2, V), dtype=BF16)
    invcnt = np.zeros((V, NB), dtype=np.float32)
    hsel = np.zeros((V, NB, R), dtype=BF16)
    tsel = np.zeros((V, NB, R), dtype=BF16)
    eht = np.empty((DIS, NB, R), dtype=BF16)
    ett = np.empty((DIS, NB, R), dtype=BF16)
    for j, b in enumerate(bs):
        pt = np.zeros((NS, V), dtype=np.float32)
        np.add.at(pt, (vidx[b].ravel(), np.repeat(np.arange(V), C)), vmask[b].ravel().astype(np.float32))
        poolt[:, j] = pt.reshape(2, 128, V).transpose(1, 0, 2).astype(BF16)
        invcnt[:, j] = 1.0 / np.maximum(vmask[b].sum(axis=1).astype(np.float32), 1.0)
        hsel[ht[b, :, 0], j, np.arange(R)] = BF16(1.0)
        tsel[ht[b, :, 1], j, np.arange(R)] = BF16(1.0)
        eht[:, j] = dis_embed_b[dis_h[b]].T
        ett[:, j] = dis_embed_b[dis_t[b]].T

    return dict(
        sent=sent, gidx=gidx, w0m=w0mv, poolt=poolt, invcnt=invcnt,
        hsel=hsel, tsel=tsel, eht=eht, ett=ett,
        w1=w1_p, w2=w2_p, b2t=b2_f,
    )


def run(trace=False, **inputs):
    global _NC_CACHE
    sentence_repr = np.asarray(inputs["sentence_repr"], dtype=np.float32)
    esi = np.asarray(inputs["entity_span_indices"]).astype(np.int64)
    vidx = np.asarray(inputs["vertex_indices"]).astype(np.int64)
    vmask = np.asarray(inputs["vertex_indices_mask"]).astype(np.int64)
    ht = np.asarray(inputs["head_tail_indices"]).astype(np.int64)
    dis_h = np.asarray(inputs["dis_h_2_t"]).astype(np.int64)
    dis_t = np.asarray(inputs["dis_t_2_h"]).astype(np.int64)
    dis_embed = np.asarray(inputs["dis_embed"], dtype=np.float32)
    w1 = np.asarray(inputs["W1"], dtype=np.float32)
    w2 = np.asarray(inputs["W2"], dtype=np.float32)
    b2 = np.asarray(inputs["b2"], dtype=np.float32)

    dis_embed_b = dis_embed.astype(BF16)
    # zero-pad W1 blocks to 20 uniform 128-row chunks, laid out [p, chunk, :]
    w1_pad = np.zeros((W1PAD, HID), dtype=BF16)
    dst = 0
    for r0, r1 in FEAT_BLOCKS:
        rows = r1 - r0
        nch = (rows + 127) // 128
        for i in range(nch):
            a = r0 + i * 128
            n = min(128, r1 - a)
            w1_pad[dst : dst + n] = w1[a : a + n].astype(BF16)
            dst += 128
    assert dst == W1PAD
    w1_p = np.ascontiguousarray(w1_pad.reshape(NKC, 128, HID).transpose(1, 0, 2))
    w2_p = np.ascontiguousarray(w2.astype(BF16).reshape(HID // 128, 128, REL).transpose(1, 0, 2))
    b2_f = b2.reshape(REL, 1).astype(np.float32)

    in_maps = [
        _prep_core(c, sentence_repr, esi, vidx, vmask, ht, dis_h, dis_t,
                   dis_embed_b, w1_p, w2_p, b2_f)
        for c in range(NCORES)
    ]

    if _NC_CACHE is None:
        _NC_CACHE = _build()

    res = bass_utils.run_bass_kernel_spmd(
        _NC_CACHE, in_maps, core_ids=list(range(NCORES)), trace=trace
    )

    out = np.empty((B, R, REL), dtype=np.float32)
    for c in range(NCORES):
        o = np.asarray(res.results[c]["outd"], dtype=np.float32)  # (NB, REL, R)
        for j in range(NB):
            out[c * NB + j] = o[j].T
    return out, res


def kernel(**inputs):
    out, _ = run(**inputs)
    return out
